# revision 21
# baseline (speedup 1.0000x reference)
"""Trainium2 Bass kernel for nn_EpistaticMultiDecoder.

Computes: adapter FFN on (1000,32) features, then for all 1e6 pairs (i,j):
head(LN -> FFN -> Linear(32,1)) of f[i]+f[j], plus ddg[i]+ddg[j].

Strategy: shard rows i across 8 cores (125 rows each, padded to 128).
On-chip layout is "x4": SBUF tile (128, N) where partition = band*32 + d
(4 bands of 32 dims), band b holds j-tokens [250b, 250b+250).

The adapter FFN and every affine per-token transform is host-precomputed in
fp64: fc (centered adapter output), FB = fc @ (hg*fw1), GB = centered(fc*hg).
Since LN1's scale r1 is constant across the 32 dims of a band, the W1 and
residual matmuls commute with the r1 multiply, so per-pair work needs only
4 matmuls: band-variance (ones), W2, and two 4-row finals (sum(yc^2), c.yc).
The finals accumulate 8 iterations into disjoint 4-partition slices of one
(64,500) PSUM tile, so the tail (rsqrt, scale, +ddg, DMA) runs once per 8
iterations on packed (32,500) tiles. All LN sqrt/div pairs are single Rsqrt
activations; Rsqrt/Relu/Square/Identity live in one activation table.
"""

import sys

sys.path.insert(0, "/opt/trn_rl_repo")

import numpy as np

B, L, A, D = 1, 50, 20, 32
M = L * A            # 1000 mutation tokens
NB = 4               # partition bands
BW = M // NB         # 250 tokens per band
NCORES = 8
RPC = M // NCORES    # 125 rows per core
RPAD = 128           # padded row count (rows 125..127 duplicate row 124)
NIT = RPAD // 2      # 64 iterations, 2 rows each
KSUP = 8             # iterations packed per "super" (16 rows)
NSUP = NIT // KSUP   # 8 supers
EPS = 1e-5

_CACHE = {}

# packed constant layouts: fp32 wpack + fp16 hpack
_worder = [("fcI", RPAD), ("fIb", RPAD), ("gI", RPAD),
           ("ddgjb", BW), ("vecs", 5), ("ddgiP", 2 * NSUP)]
WOFF = {}
_o = 0
for _n, _w in _worder:
    WOFF[_n] = (_o, _w)
    _o += _w
WPW = _o
_horder = [("x4", BW), ("f4b", BW), ("g4", BW), ("onesbd", 128), ("fbd2c", 128),
           ("wfinA", 128 * KSUP), ("wfinB", 128 * KSUP), ("g44", 2 * BW),
           ("ibd", 128)]
HOFF = {}
_o = 0
for _n, _w in _horder:
    HOFF[_n] = (_o, _w)
    _o += _w
HPW = _o


def _build_program(fast=True):
    from concourse import bacc, mybir
    from concourse.tile import TileContext

    fp32 = mybir.dt.float32
    fp32r = mybir.dt.float32r
    fp16 = mybir.dt.float16
    AF = mybir.ActivationFunctionType
    OP = mybir.AluOpType
    r = lambda ap: ap.bitcast(fp32r)

    nc = bacc.Bacc()

    def act_raw(out, in_, func, bias, scale):
        # nc.scalar.activation body minus the Rsqrt accuracy guard
        eng = nc.scalar
        ins = [eng.lower_ap(in_), eng.lower_ap(bias),
               mybir.ImmediateValue(dtype=fp32, value=scale),
               mybir.ImmediateValue(dtype=fp32, value=0.0)]
        return eng.add_instruction(mybir.InstActivation(
            name=nc.get_next_instruction_name(), func=func,
            ins=ins, outs=[eng.lower_ap(out)]))

    wp = nc.dram_tensor("wpack", [128, WPW], fp32, kind="ExternalInput")
    hp = nc.dram_tensor("hpack", [128, HPW], fp16, kind="ExternalInput")
    out_d = nc.dram_tensor("out", [RPC, M], fp32, kind="ExternalOutput")

    with TileContext(nc) as tc:
        with (
            tc.tile_pool(name="consts", bufs=1) as consts,
            tc.tile_pool(name="lp", bufs=3) as lp,
            tc.tile_pool(name="sup", bufs=2) as sup,
            tc.tile_pool(name="psA", bufs=3, space="PSUM") as psA,
            tc.tile_pool(name="psC", bufs=3, space="PSUM") as psC,
            tc.tile_pool(name="ps64", bufs=2, space="PSUM") as ps64p,
        ):
            wpack = consts.tile_from(wp[:, :], name="wpack")
            hpack = consts.tile_from(hp[:, :], name="hpack")
            ct = {k: wpack[:, o:o + w] for k, (o, w) in WOFF.items()}
            ct.update({k: hpack[:, o:o + w] for k, (o, w) in HOFF.items()})
            x4 = ct["x4"]
            vecs = ct["vecs"]
            fb1r = vecs[:, 0:1]    # tile4(fb1 + hbt @ fw1)
            ycb = vecs[:, 1:2]     # tile4((fb2 + hbt) @ C32)
            epsr = vecs[:, 2:3]    # EPS
            epsK = vecs[:, 3:4]    # EPS + sum(ycb^2)/D
            k2c = vecs[:, 4:5]     # sum(cvec * ycb)

            for s in range(NSUP):
                ps64 = ps64p.tile([128, 2 * BW], fp32, tag="ps64")
                for k in range(KSUP):
                    p2 = 2 * (s * KSUP + k)

                    # LN1 variance: sum over band of (fc_i + fc_j)^2
                    pairc = lp.tile([128, 2 * BW], fp16, tag="pairc")
                    for h in range(2):
                        nc.vector.tensor_scalar_add(
                            pairc[:, h * BW:(h + 1) * BW], x4,
                            ct["fcI"][:, p2 + h:p2 + h + 1])
                    pairsq = lp.tile([128, 2 * BW], fp16, tag="pairsq")
                    nc.gpsimd.tensor_mul(pairsq, pairc, pairc)
                    psa = psA.tile([128, 2 * BW], fp32, tag="psA")
                    nc.tensor.matmul(psa, ct["onesbd"], pairsq,
                                     start=True, stop=True)
                    r1 = lp.tile([128, 2 * BW], fp16, tag="r1")
                    act_raw(r1, psa, AF.Rsqrt, epsr, 1.0 / D)

                    if fast:
                        # fb1' == 0: relu commutes with the r1 scale and the
                        # block-diag W2 matmul. hrhat = relu(FB_i + FB_j),
                        # psc = W2 @ hrhat + (GB_i replicated + GB_j), and
                        # u = (psc + GI_col) * r1; true yc = u + ycb with ycb
                        # folded into the final contraction weights.
                        hrh = lp.tile([128, 2 * BW], fp16, tag="hrh")
                        for h in range(2):
                            nc.scalar.activation(
                                hrh[:, h * BW:(h + 1) * BW], ct["f4b"],
                                AF.Relu,
                                bias=ct["fIb"][:, p2 + h:p2 + h + 1],
                                scale=1.0)
                        psc = psC.tile([128, 2 * BW], fp32, tag="psC")
                        nc.tensor.matmul(psc, ct["fbd2c"], hrh,
                                         start=True, stop=False)
                        nc.tensor.matmul(psc, ct["ibd"], ct["g44"],
                                         start=False, stop=True)
                        u = lp.tile([128, 2 * BW], fp16, tag="u")
                        for h in range(2):
                            sl = slice(h * BW, (h + 1) * BW)
                            nc.vector.scalar_tensor_tensor(
                                u[:, sl], psc[:, sl],
                                ct["gI"][:, p2 + h:p2 + h + 1], r1[:, sl],
                                op0=OP.add, op1=OP.mult)
                        usq = lp.tile([128, 2 * BW], fp16, tag="usq")
                        nc.scalar.activation(usq, u, AF.Square,
                                             bias=0.0, scale=1.0)
                        nc.tensor.matmul(ps64,
                                         ct["wfinA"][:, 128 * k:128 * k + 128],
                                         usq, start=(k == 0), stop=False)
                        nc.tensor.matmul(ps64,
                                         ct["wfinB"][:, 128 * k:128 * k + 128],
                                         u, start=False, stop=(k == KSUP - 1))
                    else:
                        rin = lp.tile([128, 2 * BW], fp16, tag="rin")
                        for h in range(2):
                            sl = slice(h * BW, (h + 1) * BW)
                            nc.vector.scalar_tensor_tensor(
                                rin[:, sl], ct["f4b"],
                                ct["fIb"][:, p2 + h:p2 + h + 1], r1[:, sl],
                                op0=OP.add, op1=OP.mult)
                        hr = lp.tile([128, 2 * BW], fp16, tag="hr")
                        nc.scalar.activation(hr, rin, AF.Relu,
                                             bias=fb1r, scale=1.0)
                        psc = psC.tile([128, 2 * BW], fp32, tag="psC")
                        nc.tensor.matmul(psc, ct["fbd2c"], hr,
                                         start=True, stop=True)
                        gsc = lp.tile([128, 2 * BW], fp16, tag="gsc")
                        for h in range(2):
                            sl = slice(h * BW, (h + 1) * BW)
                            nc.vector.scalar_tensor_tensor(
                                gsc[:, sl], ct["g4"],
                                ct["gI"][:, p2 + h:p2 + h + 1], r1[:, sl],
                                op0=OP.add, op1=OP.mult)
                        yc = lp.tile([128, 2 * BW], fp16, tag="yc")
                        nc.vector.scalar_tensor_tensor(
                            yc, psc, ycb, gsc, op0=OP.add, op1=OP.add)
                        ysq = lp.tile([128, 2 * BW], fp16, tag="ysq")
                        nc.gpsimd.tensor_mul(ysq, yc, yc)
                        nc.tensor.matmul(ps64,
                                         ct["wfinA"][:, 128 * k:128 * k + 128],
                                         ysq, start=(k == 0), stop=False)
                        nc.tensor.matmul(ps64,
                                         ct["wfinB"][:, 128 * k:128 * k + 128],
                                         yc, start=False, stop=(k == KSUP - 1))

                # super tail: 16 rows of output from packed scalars
                r2p = sup.tile([32, 2 * BW], fp32, tag="r2p")
                act_raw(r2p, ps64[0:32, :], AF.Rsqrt, epsK[0:32, :], 1.0 / D)
                tmp = sup.tile([32, 2 * BW], fp32, tag="tmp")
                nc.vector.scalar_tensor_tensor(
                    tmp, ps64[32:64, :], k2c[0:32, :], r2p,
                    op0=OP.add, op1=OP.mult)
                orow = sup.tile([32, 2 * BW], fp32, tag="orow")
                for h in range(2):
                    sl = slice(h * BW, (h + 1) * BW)
                    nc.vector.scalar_tensor_tensor(
                        orow[:, sl], tmp[:, sl],
                        ct["ddgiP"][0:32, 2 * s + h:2 * s + h + 1],
                        ct["ddgjb"][0:32, :], op0=OP.add, op1=OP.add)

                nk_full = KSUP if s < NSUP - 1 else 6
                for k in range(nk_full):
                    i0 = 16 * s + 2 * k
                    nc.sync.dma_start(
                        out_d[i0:i0 + 2, :].rearrange("h (b c) -> b h c", b=NB),
                        orow[4 * k:4 * k + 4, :].rearrange(
                            "b (h c) -> b h c", h=2))
                if s == NSUP - 1:
                    # row 124 = the half iteration (k=6, h=0)
                    nc.sync.dma_start(
                        out_d[124:125, :].rearrange("o (b c) -> (o b) c", b=NB),
                        orow[24:28, 0:BW])

    nc.compile()
    return nc


def _x4(mat):
    """(1000, 32) -> (128, 250): partition band*32+d, free = token-in-band."""
    return np.ascontiguousarray(
        mat.reshape(NB, BW, D).transpose(0, 2, 1).reshape(128, BW))


def _host_prep(mut1_feat, mut1_ddg, aw1, ab1, aw2, ab2, ag, abt,
               hg, hbt, fw1, fb1, fw2, fb2, fg, fbt, ow, ob):
    f32 = np.float32
    f64 = np.float64
    C = np.eye(D) - np.ones((D, D)) / D
    bd = lambda m: np.kron(np.eye(NB), m).astype(f32)
    tile4 = lambda v: np.tile(np.asarray(v, f64), NB).astype(f32)

    fm = np.asarray(mut1_feat, f64).reshape(M, D)
    aw1_, aw2_, fw1_, fw2_, ow_ = [np.asarray(a, f64) for a in
                                   (aw1, aw2, fw1, fw2, ow)]
    ag_, abt_, hg_, hbt_, fg_, fbt_, ab1_, ab2_, fb1_, fb2_, ob_ = [
        np.asarray(a, f64) for a in
        (ag, abt, hg, hbt, fg, fbt, ab1, ab2, fb1, fb2, ob)]

    # adapter FFN (exact, host fp64), then center so pair sums are centered
    h1 = np.maximum(fm @ aw1_ + ab1_, 0.0)
    y = fm + h1 @ aw2_ + ab2_
    mu = y.mean(-1, keepdims=True)
    v = ((y - mu) ** 2).mean(-1, keepdims=True)
    f = (y - mu) / np.sqrt(v + EPS) * ag_ + abt_
    fc = f - f.mean(-1, keepdims=True)                  # (1000, 32)

    FB = fc @ (hg_[:, None] * fw1_)                     # W1 path, pre-r1
    gz = fc * hg_
    GB = gz - gz.mean(-1, keepdims=True)                # residual path, pre-r1

    cvec = (fg_ * ow_[:, 0])                            # folded LN2 + linear
    fb1r_v = fb1_ + hbt_ @ fw1_
    fast = bool(np.all(fb1r_v == 0.0))
    ycb_v = (fb2_ + hbt_) @ C
    K1 = float(ycb_v @ ycb_v)
    K2 = float(cvec @ ycb_v)
    wfinA = np.zeros((128, 128 * KSUP), f32)
    wfinB = np.zeros((128, 128 * KSUP), f32)
    for kk in range(KSUP):
        for b in range(NB):
            wfinA[32 * b:32 * b + 32, 128 * kk + 4 * kk + b] = 1.0
            wfinB[32 * b:32 * b + 32, 128 * kk + 32 + 4 * kk + b] = cvec
            if fast:
                # cross term 2*ycb.u of sum(yc^2), yc = u + ycb
                wfinB[32 * b:32 * b + 32, 128 * kk + 4 * kk + b] = 2.0 * ycb_v
    obp = float(ob_[0] + fbt_ @ ow_[:, 0])

    ddg = np.asarray(mut1_ddg, f64).reshape(M)
    ddgjb = np.zeros((128, BW), f32)
    for kk in range(KSUP):
        for b in range(NB):
            ddgjb[4 * kk + b, :] = (ddg[BW * b:BW * (b + 1)] + obp)

    f16 = np.float16
    g4x = _x4(GB)
    hbase = {
        "onesbd": bd(np.ones((D, D))).astype(f16),
        "fbd2c": bd(fw2_ @ C).astype(f16),
        "wfinA": wfinA.astype(f16),
        "wfinB": wfinB.astype(f16),
        "x4": _x4(fc).astype(f16),
        "f4b": _x4(FB).astype(f16),
        "g4": g4x.astype(f16),
        "g44": np.concatenate([g4x, g4x], axis=1).astype(f16),
        "ibd": np.eye(128).astype(f16),
    }
    hpack = np.zeros((128, HPW), f16)
    for kname, (o, w) in HOFF.items():
        hpack[:, o:o + w] = hbase[kname]
    base = {
        "ddgjb": ddgjb,
        "vecs": np.stack([
            tile4(fb1r_v),
            tile4(ycb_v),
            np.full(128, EPS),
            np.full(128, EPS + (K1 / D if fast else 0.0)),
            np.full(128, K2 if fast else 0.0),
        ], axis=1).astype(f32),
    }

    in_maps = []
    for c in range(NCORES):
        r0 = c * RPC
        ridx = np.minimum(np.arange(RPAD) + r0, r0 + RPC - 1)
        ddgiP = np.zeros((128, 2 * NSUP), f32)
        for s in range(NSUP):
            for hh in range(2):
                for kk in range(KSUP):
                    for b in range(NB):
                        ddgiP[4 * kk + b, 2 * s + hh] = ddg[
                            ridx[16 * s + 2 * kk + hh]]
        wpack = np.zeros((128, WPW), f32)
        for name, mat in (("fcI", fc), ("fIb", FB), ("gI", GB)):
            o, w = WOFF[name]
            col = np.tile(mat[ridx].T, (NB, 1)).astype(f32)   # (128, 128)
            wpack[:, o:o + w] = col
        for kname in ("ddgjb", "vecs"):
            o, w = WOFF[kname]
            wpack[:, o:o + w] = base[kname]
        o, w = WOFF["ddgiP"]
        wpack[:, o:o + w] = ddgiP
        in_maps.append({"wpack": wpack, "hpack": hpack})
    return (in_maps, fast)


def _sl(name):
    o, w = WOFF[name]
    return o, o + w


def _run(prep, **kw):
    from concourse.bass_utils import run_bass_kernel_spmd
    in_maps, fast = prep
    key = "nc_fast" if fast else "nc_slow"
    if key not in _CACHE:
        _CACHE[key] = _build_program(fast=fast)
    return run_bass_kernel_spmd(_CACHE[key], in_maps,
                                core_ids=list(range(NCORES)), **kw)


def kernel(**inputs):
    res = _run(_host_prep(**inputs))
    rows = np.concatenate([res.results[c]["out"] for c in range(NCORES)], axis=0)
    return rows.reshape(B, L, A, L, A).astype(np.float32)


# revision 22
# speedup vs baseline: 1.0975x; 1.0975x over previous
"""Trainium2 Bass kernel for nn_EpistaticMultiDecoder.

Computes: adapter FFN on (1000,32) features, then for all 1e6 pairs (i,j):
head(LN -> FFN -> Linear(32,1)) of f[i]+f[j], plus ddg[i]+ddg[j].

Strategy: shard rows i across 8 cores (125 rows each, padded to 128).
On-chip layout is "x4": SBUF tile (128, N) where partition = band*32 + d
(4 bands of 32 dims), band b holds j-tokens [250b, 250b+250).

The adapter FFN and every affine per-token transform is host-precomputed in
fp64: fc (centered adapter output), FB = fc @ (hg*fw1), GB = centered(fc*hg).
Since LN1's scale r1 is constant across the 32 dims of a band, the W1 and
residual matmuls commute with the r1 multiply, so per-pair work needs only
4 matmuls: band-variance (ones), W2, and two 4-row finals (sum(yc^2), c.yc).
The finals accumulate 8 iterations into disjoint 4-partition slices of one
(64,500) PSUM tile, so the tail (rsqrt, scale, +ddg, DMA) runs once per 8
iterations on packed (32,500) tiles. All LN sqrt/div pairs are single Rsqrt
activations; Rsqrt/Relu/Square/Identity live in one activation table.
"""

import sys

sys.path.insert(0, "/opt/trn_rl_repo")

import numpy as np

B, L, A, D = 1, 50, 20, 32
M = L * A            # 1000 mutation tokens
NB = 4               # partition bands
BW = M // NB         # 250 tokens per band
NCORES = 8
RPC = M // NCORES    # 125 rows per core
RPAD = 128           # padded row count (rows 125..127 duplicate row 124)
NIT = RPAD // 2      # 64 iterations, 2 rows each
KSUP = 8             # iterations packed per "super" (16 rows)
NSUP = NIT // KSUP   # 8 supers
EPS = 1e-5

_CACHE = {}

# packed constant layouts: fp32 wpack + fp16 hpack
_worder = [("fcI", RPAD), ("fIb", RPAD), ("gI", RPAD),
           ("ddgjb", BW), ("vecs", 5), ("ddgiP", 2 * NSUP)]
WOFF = {}
_o = 0
for _n, _w in _worder:
    WOFF[_n] = (_o, _w)
    _o += _w
WPW = _o
_horder = [("x4", BW), ("f4b", BW), ("g4", BW), ("onesbd", 128), ("fbd2c", 128),
           ("wfinA", 128 * KSUP), ("wfinB", 128 * KSUP), ("g44", 2 * BW),
           ("ibd", 128)]
HOFF = {}
_o = 0
for _n, _w in _horder:
    HOFF[_n] = (_o, _w)
    _o += _w
HPW = _o


def _build_program(fast=True):
    from concourse import bacc, mybir
    from concourse.tile import TileContext

    fp32 = mybir.dt.float32
    fp32r = mybir.dt.float32r
    fp16 = mybir.dt.float16
    AF = mybir.ActivationFunctionType
    OP = mybir.AluOpType
    r = lambda ap: ap.bitcast(fp32r)

    nc = bacc.Bacc()

    def act_raw(out, in_, func, bias, scale):
        # nc.scalar.activation body minus the Rsqrt accuracy guard
        eng = nc.scalar
        ins = [eng.lower_ap(in_), eng.lower_ap(bias),
               mybir.ImmediateValue(dtype=fp32, value=scale),
               mybir.ImmediateValue(dtype=fp32, value=0.0)]
        return eng.add_instruction(mybir.InstActivation(
            name=nc.get_next_instruction_name(), func=func,
            ins=ins, outs=[eng.lower_ap(out)]))

    wp = nc.dram_tensor("wpack", [128, WPW], fp32, kind="ExternalInput")
    hp = nc.dram_tensor("hpack", [128, HPW], fp16, kind="ExternalInput")
    out_d = nc.dram_tensor("out", [RPC, M], fp32, kind="ExternalOutput")

    with TileContext(nc) as tc:
        with (
            tc.tile_pool(name="consts", bufs=1) as consts,
            tc.tile_pool(name="lp", bufs=3) as lp,
            tc.tile_pool(name="sup", bufs=2) as sup,
            tc.tile_pool(name="psA", bufs=3, space="PSUM") as psA,
            tc.tile_pool(name="psC", bufs=3, space="PSUM") as psC,
            tc.tile_pool(name="ps64", bufs=2, space="PSUM") as ps64p,
        ):
            wpack = consts.tile_from(wp[:, :], name="wpack")
            hpack = consts.tile_from(hp[:, :], name="hpack")
            ct = {k: wpack[:, o:o + w] for k, (o, w) in WOFF.items()}
            ct.update({k: hpack[:, o:o + w] for k, (o, w) in HOFF.items()})
            x4 = ct["x4"]
            vecs = ct["vecs"]
            fb1r = vecs[:, 0:1]    # tile4(fb1 + hbt @ fw1)
            ycb = vecs[:, 1:2]     # tile4((fb2 + hbt) @ C32)
            epsr = vecs[:, 2:3]    # EPS
            epsK = vecs[:, 3:4]    # EPS + sum(ycb^2)/D
            k2c = vecs[:, 4:5]     # sum(cvec * ycb)

            for s in range(NSUP):
                ps64 = ps64p.tile([128, 2 * BW], fp32, tag="ps64")
                for k in range(KSUP):
                    p2 = 2 * (s * KSUP + k)

                    # LN1 variance: sum over band of (fc_i + fc_j)^2
                    pairc = lp.tile([128, 2 * BW], fp16, tag="pairc")
                    for h in range(2):
                        nc.vector.tensor_scalar_add(
                            pairc[:, h * BW:(h + 1) * BW], x4,
                            ct["fcI"][:, p2 + h:p2 + h + 1])
                    pairsq = lp.tile([128, 2 * BW], fp16, tag="pairsq")
                    nc.gpsimd.tensor_mul(pairsq, pairc, pairc)
                    psa = psA.tile([128, 2 * BW], fp32, tag="psA")
                    nc.tensor.matmul(psa, ct["onesbd"], pairsq,
                                     start=True, stop=True)
                    r1 = lp.tile([128, 2 * BW], fp16, tag="r1")
                    act_raw(r1, psa, AF.Rsqrt, epsr, 1.0 / D)

                    if fast:
                        # fb1' == 0: relu commutes with the r1 scale and the
                        # block-diag W2 matmul. hrhat = relu(FB_i + FB_j),
                        # psc = W2 @ hrhat + (GB_i replicated + GB_j), and
                        # u = (psc + GI_col) * r1; true yc = u + ycb with ycb
                        # folded into the final contraction weights.
                        hrh = lp.tile([128, 2 * BW], fp16, tag="hrh")
                        for h in range(2):
                            nc.scalar.activation(
                                hrh[:, h * BW:(h + 1) * BW], ct["f4b"],
                                AF.Relu,
                                bias=ct["fIb"][:, p2 + h:p2 + h + 1],
                                scale=1.0)
                        psc = psC.tile([128, 2 * BW], fp32, tag="psC")
                        nc.tensor.matmul(psc, ct["fbd2c"], hrh,
                                         start=True, stop=False)
                        nc.tensor.matmul(psc, ct["ibd"], ct["g44"],
                                         start=False, stop=True)
                        u = lp.tile([128, 2 * BW], fp16, tag="u")
                        for h in range(2):
                            sl = slice(h * BW, (h + 1) * BW)
                            nc.vector.scalar_tensor_tensor(
                                u[:, sl], psc[:, sl],
                                ct["gI"][:, p2 + h:p2 + h + 1], r1[:, sl],
                                op0=OP.add, op1=OP.mult)
                        usq = lp.tile([128, 2 * BW], fp16, tag="usq")
                        nc.gpsimd.tensor_mul(usq, u, u)
                        nc.tensor.matmul(ps64,
                                         ct["wfinA"][:, 128 * k:128 * k + 128],
                                         usq, start=(k == 0), stop=False)
                        nc.tensor.matmul(ps64,
                                         ct["wfinB"][:, 128 * k:128 * k + 128],
                                         u, start=False, stop=(k == KSUP - 1))
                    else:
                        rin = lp.tile([128, 2 * BW], fp16, tag="rin")
                        for h in range(2):
                            sl = slice(h * BW, (h + 1) * BW)
                            nc.vector.scalar_tensor_tensor(
                                rin[:, sl], ct["f4b"],
                                ct["fIb"][:, p2 + h:p2 + h + 1], r1[:, sl],
                                op0=OP.add, op1=OP.mult)
                        hr = lp.tile([128, 2 * BW], fp16, tag="hr")
                        nc.scalar.activation(hr, rin, AF.Relu,
                                             bias=fb1r, scale=1.0)
                        psc = psC.tile([128, 2 * BW], fp32, tag="psC")
                        nc.tensor.matmul(psc, ct["fbd2c"], hr,
                                         start=True, stop=True)
                        gsc = lp.tile([128, 2 * BW], fp16, tag="gsc")
                        for h in range(2):
                            sl = slice(h * BW, (h + 1) * BW)
                            nc.vector.scalar_tensor_tensor(
                                gsc[:, sl], ct["g4"],
                                ct["gI"][:, p2 + h:p2 + h + 1], r1[:, sl],
                                op0=OP.add, op1=OP.mult)
                        yc = lp.tile([128, 2 * BW], fp16, tag="yc")
                        nc.vector.scalar_tensor_tensor(
                            yc, psc, ycb, gsc, op0=OP.add, op1=OP.add)
                        ysq = lp.tile([128, 2 * BW], fp16, tag="ysq")
                        nc.gpsimd.tensor_mul(ysq, yc, yc)
                        nc.tensor.matmul(ps64,
                                         ct["wfinA"][:, 128 * k:128 * k + 128],
                                         ysq, start=(k == 0), stop=False)
                        nc.tensor.matmul(ps64,
                                         ct["wfinB"][:, 128 * k:128 * k + 128],
                                         yc, start=False, stop=(k == KSUP - 1))

                # super tail: 16 rows of output from packed scalars
                r2p = sup.tile([32, 2 * BW], fp32, tag="r2p")
                act_raw(r2p, ps64[0:32, :], AF.Rsqrt, epsK[0:32, :], 1.0 / D)
                tmp = sup.tile([32, 2 * BW], fp32, tag="tmp")
                nc.vector.scalar_tensor_tensor(
                    tmp, ps64[32:64, :], k2c[0:32, :], r2p,
                    op0=OP.add, op1=OP.mult)
                orow = sup.tile([32, 2 * BW], fp32, tag="orow")
                for h in range(2):
                    sl = slice(h * BW, (h + 1) * BW)
                    nc.vector.scalar_tensor_tensor(
                        orow[:, sl], tmp[:, sl],
                        ct["ddgiP"][0:32, 2 * s + h:2 * s + h + 1],
                        ct["ddgjb"][0:32, :], op0=OP.add, op1=OP.add)

                nk_full = KSUP if s < NSUP - 1 else 6
                for k in range(nk_full):
                    i0 = 16 * s + 2 * k
                    nc.sync.dma_start(
                        out_d[i0:i0 + 2, :].rearrange("h (b c) -> b h c", b=NB),
                        orow[4 * k:4 * k + 4, :].rearrange(
                            "b (h c) -> b h c", h=2))
                if s == NSUP - 1:
                    # row 124 = the half iteration (k=6, h=0)
                    nc.sync.dma_start(
                        out_d[124:125, :].rearrange("o (b c) -> (o b) c", b=NB),
                        orow[24:28, 0:BW])

    nc.compile()
    return nc


def _x4(mat):
    """(1000, 32) -> (128, 250): partition band*32+d, free = token-in-band."""
    return np.ascontiguousarray(
        mat.reshape(NB, BW, D).transpose(0, 2, 1).reshape(128, BW))


def _host_prep(mut1_feat, mut1_ddg, aw1, ab1, aw2, ab2, ag, abt,
               hg, hbt, fw1, fb1, fw2, fb2, fg, fbt, ow, ob):
    f32 = np.float32
    f64 = np.float64
    C = np.eye(D) - np.ones((D, D)) / D
    bd = lambda m: np.kron(np.eye(NB), m).astype(f32)
    tile4 = lambda v: np.tile(np.asarray(v, f64), NB).astype(f32)

    fm = np.asarray(mut1_feat, f64).reshape(M, D)
    aw1_, aw2_, fw1_, fw2_, ow_ = [np.asarray(a, f64) for a in
                                   (aw1, aw2, fw1, fw2, ow)]
    ag_, abt_, hg_, hbt_, fg_, fbt_, ab1_, ab2_, fb1_, fb2_, ob_ = [
        np.asarray(a, f64) for a in
        (ag, abt, hg, hbt, fg, fbt, ab1, ab2, fb1, fb2, ob)]

    # adapter FFN (exact, host fp64), then center so pair sums are centered
    h1 = np.maximum(fm @ aw1_ + ab1_, 0.0)
    y = fm + h1 @ aw2_ + ab2_
    mu = y.mean(-1, keepdims=True)
    v = ((y - mu) ** 2).mean(-1, keepdims=True)
    f = (y - mu) / np.sqrt(v + EPS) * ag_ + abt_
    fc = f - f.mean(-1, keepdims=True)                  # (1000, 32)

    FB = fc @ (hg_[:, None] * fw1_)                     # W1 path, pre-r1
    gz = fc * hg_
    GB = gz - gz.mean(-1, keepdims=True)                # residual path, pre-r1

    cvec = (fg_ * ow_[:, 0])                            # folded LN2 + linear
    fb1r_v = fb1_ + hbt_ @ fw1_
    fast = bool(np.all(fb1r_v == 0.0))
    ycb_v = (fb2_ + hbt_) @ C
    K1 = float(ycb_v @ ycb_v)
    K2 = float(cvec @ ycb_v)
    wfinA = np.zeros((128, 128 * KSUP), f32)
    wfinB = np.zeros((128, 128 * KSUP), f32)
    for kk in range(KSUP):
        for b in range(NB):
            wfinA[32 * b:32 * b + 32, 128 * kk + 4 * kk + b] = 1.0
            wfinB[32 * b:32 * b + 32, 128 * kk + 32 + 4 * kk + b] = cvec
            if fast:
                # cross term 2*ycb.u of sum(yc^2), yc = u + ycb
                wfinB[32 * b:32 * b + 32, 128 * kk + 4 * kk + b] = 2.0 * ycb_v
    obp = float(ob_[0] + fbt_ @ ow_[:, 0])

    ddg = np.asarray(mut1_ddg, f64).reshape(M)
    ddgjb = np.zeros((128, BW), f32)
    for kk in range(KSUP):
        for b in range(NB):
            ddgjb[4 * kk + b, :] = (ddg[BW * b:BW * (b + 1)] + obp)

    f16 = np.float16
    g4x = _x4(GB)
    hbase = {
        "onesbd": bd(np.ones((D, D))).astype(f16),
        "fbd2c": bd(fw2_ @ C).astype(f16),
        "wfinA": wfinA.astype(f16),
        "wfinB": wfinB.astype(f16),
        "x4": _x4(fc).astype(f16),
        "f4b": _x4(FB).astype(f16),
        "g4": g4x.astype(f16),
        "g44": np.concatenate([g4x, g4x], axis=1).astype(f16),
        "ibd": np.eye(128).astype(f16),
    }
    hpack = np.zeros((128, HPW), f16)
    for kname, (o, w) in HOFF.items():
        hpack[:, o:o + w] = hbase[kname]
    base = {
        "ddgjb": ddgjb,
        "vecs": np.stack([
            tile4(fb1r_v),
            tile4(ycb_v),
            np.full(128, EPS),
            np.full(128, EPS + (K1 / D if fast else 0.0)),
            np.full(128, K2 if fast else 0.0),
        ], axis=1).astype(f32),
    }

    in_maps = []
    for c in range(NCORES):
        r0 = c * RPC
        ridx = np.minimum(np.arange(RPAD) + r0, r0 + RPC - 1)
        ddgiP = np.zeros((128, 2 * NSUP), f32)
        for s in range(NSUP):
            for hh in range(2):
                for kk in range(KSUP):
                    for b in range(NB):
                        ddgiP[4 * kk + b, 2 * s + hh] = ddg[
                            ridx[16 * s + 2 * kk + hh]]
        wpack = np.zeros((128, WPW), f32)
        for name, mat in (("fcI", fc), ("fIb", FB), ("gI", GB)):
            o, w = WOFF[name]
            col = np.tile(mat[ridx].T, (NB, 1)).astype(f32)   # (128, 128)
            wpack[:, o:o + w] = col
        for kname in ("ddgjb", "vecs"):
            o, w = WOFF[kname]
            wpack[:, o:o + w] = base[kname]
        o, w = WOFF["ddgiP"]
        wpack[:, o:o + w] = ddgiP
        in_maps.append({"wpack": wpack, "hpack": hpack})
    return (in_maps, fast)


def _sl(name):
    o, w = WOFF[name]
    return o, o + w


def _run(prep, **kw):
    from concourse.bass_utils import run_bass_kernel_spmd
    in_maps, fast = prep
    key = "nc_fast" if fast else "nc_slow"
    if key not in _CACHE:
        _CACHE[key] = _build_program(fast=fast)
    return run_bass_kernel_spmd(_CACHE[key], in_maps,
                                core_ids=list(range(NCORES)), **kw)


def kernel(**inputs):
    res = _run(_host_prep(**inputs))
    rows = np.concatenate([res.results[c]["out"] for c in range(NCORES)], axis=0)
    return rows.reshape(B, L, A, L, A).astype(np.float32)


# revision 23
# speedup vs baseline: 1.3165x; 1.1995x over previous
"""Trainium2 Bass kernel for nn_EpistaticMultiDecoder.

Computes: adapter FFN on (1000,32) features, then for all 1e6 pairs (i,j):
head(LN -> FFN -> Linear(32,1)) of f[i]+f[j], plus ddg[i]+ddg[j].

Strategy: shard rows i across 8 cores (125 rows each, padded to 128).
On-chip layout is "x4": SBUF tile (128, N) where partition = band*32 + d
(4 bands of 32 dims), band b holds j-tokens [250b, 250b+250).

The adapter FFN and every affine per-token transform is host-precomputed in
fp64: fc (centered adapter output), FB = fc @ (hg*fw1), GB = centered(fc*hg).
Since LN1's scale r1 is constant across the 32 dims of a band, the W1 and
residual matmuls commute with the r1 multiply, so per-pair work needs only
4 matmuls: band-variance (ones), W2, and two 4-row finals (sum(yc^2), c.yc).
The finals accumulate 8 iterations into disjoint 4-partition slices of one
(64,500) PSUM tile, so the tail (rsqrt, scale, +ddg, DMA) runs once per 8
iterations on packed (32,500) tiles. All LN sqrt/div pairs are single Rsqrt
activations; Rsqrt/Relu/Square/Identity live in one activation table.
"""

import sys

sys.path.insert(0, "/opt/trn_rl_repo")

import numpy as np

B, L, A, D = 1, 50, 20, 32
M = L * A            # 1000 mutation tokens
NB = 4               # partition bands
BW = M // NB         # 250 tokens per band
NCORES = 8
RPC = M // NCORES    # 125 rows per core
RPAD = 128           # padded row count (rows 125..127 duplicate row 124)
NIT = RPAD // 2      # 64 iterations, 2 rows each
KSUP = 8             # iterations packed per "super" (16 rows)
NSUP = NIT // KSUP   # 8 supers
EPS = 1e-5

_CACHE = {}

# packed constant layouts: fp32 wpack + fp16 hpack
_worder = [("fcI", RPAD), ("fIb", RPAD), ("gI", RPAD),
           ("ddgjb", BW), ("vecs", 5), ("ddgiP", 2 * NSUP)]
WOFF = {}
_o = 0
for _n, _w in _worder:
    WOFF[_n] = (_o, _w)
    _o += _w
WPW = _o
_horder = [("x4", BW), ("f4b", BW), ("g4", BW), ("onesbd", 128), ("fbd2c", 128),
           ("wfinA", 128 * KSUP), ("wfinB", 128 * KSUP), ("g44", 2 * BW),
           ("ibd", 128)]
HOFF = {}
_o = 0
for _n, _w in _horder:
    HOFF[_n] = (_o, _w)
    _o += _w
HPW = _o


def _build_program(fast=True):
    from concourse import bacc, mybir
    from concourse.tile import TileContext

    fp32 = mybir.dt.float32
    fp32r = mybir.dt.float32r
    fp16 = mybir.dt.float16
    AF = mybir.ActivationFunctionType
    OP = mybir.AluOpType
    r = lambda ap: ap.bitcast(fp32r)

    nc = bacc.Bacc()

    def act_raw(out, in_, func, bias, scale):
        # nc.scalar.activation body minus the Rsqrt accuracy guard
        eng = nc.scalar
        ins = [eng.lower_ap(in_), eng.lower_ap(bias),
               mybir.ImmediateValue(dtype=fp32, value=scale),
               mybir.ImmediateValue(dtype=fp32, value=0.0)]
        return eng.add_instruction(mybir.InstActivation(
            name=nc.get_next_instruction_name(), func=func,
            ins=ins, outs=[eng.lower_ap(out)]))

    wp = nc.dram_tensor("wpack", [128, WPW], fp32, kind="ExternalInput")
    hp = nc.dram_tensor("hpack", [128, HPW], fp16, kind="ExternalInput")
    out_d = nc.dram_tensor("out", [RPC, M], fp32, kind="ExternalOutput")

    with TileContext(nc) as tc:
        with (
            tc.tile_pool(name="consts", bufs=1) as consts,
            tc.tile_pool(name="lp", bufs=4) as lp,
            tc.tile_pool(name="sup", bufs=2) as sup,
            tc.tile_pool(name="psA", bufs=3, space="PSUM") as psA,
            tc.tile_pool(name="psC", bufs=3, space="PSUM") as psC,
            tc.tile_pool(name="ps64", bufs=2, space="PSUM") as ps64p,
        ):
            wpack = consts.tile_from(wp[:, :], name="wpack")
            hpack = consts.tile_from(hp[:, :], name="hpack")
            ct = {k: wpack[:, o:o + w] for k, (o, w) in WOFF.items()}
            ct.update({k: hpack[:, o:o + w] for k, (o, w) in HOFF.items()})
            x4 = ct["x4"]
            vecs = ct["vecs"]
            fb1r = vecs[:, 0:1]    # tile4(fb1 + hbt @ fw1)
            ycb = vecs[:, 1:2]     # tile4((fb2 + hbt) @ C32)
            epsr = vecs[:, 2:3]    # EPS
            epsK = vecs[:, 3:4]    # EPS + sum(ycb^2)/D
            k2c = vecs[:, 4:5]     # sum(cvec * ycb)

            for s in range(NSUP):
                ps64 = ps64p.tile([128, 2 * BW], fp32, tag="ps64")
                for k in range(KSUP):
                    p2 = 2 * (s * KSUP + k)

                    # LN1 variance: sum over band of (fc_i + fc_j)^2
                    pairsq = lp.tile([128, 2 * BW], fp16, tag="pairsq")
                    for h in range(2):
                        nc.scalar.activation(
                            pairsq[:, h * BW:(h + 1) * BW], x4, AF.Square,
                            bias=ct["fcI"][:, p2 + h:p2 + h + 1], scale=1.0)
                    psa = psA.tile([128, 2 * BW], fp32, tag="psA")
                    nc.tensor.matmul(psa, ct["onesbd"], pairsq,
                                     start=True, stop=True)
                    r1 = lp.tile([128, 2 * BW], fp16, tag="r1")
                    act_raw(r1, psa, AF.Rsqrt, epsr, 1.0 / D)

                    if fast:
                        # fb1' == 0: relu commutes with the r1 scale and the
                        # block-diag W2 matmul. hrhat = relu(FB_i + FB_j),
                        # psc = W2 @ hrhat + (GB_i replicated + GB_j), and
                        # u = (psc + GI_col) * r1; true yc = u + ycb with ycb
                        # folded into the final contraction weights.
                        hrh = lp.tile([128, 2 * BW], fp16, tag="hrh")
                        for h in range(2):
                            nc.vector.tensor_scalar(
                                hrh[:, h * BW:(h + 1) * BW], ct["f4b"],
                                ct["fIb"][:, p2 + h:p2 + h + 1], 0.0,
                                op0=OP.add, op1=OP.max)
                        psc = psC.tile([128, 2 * BW], fp32, tag="psC")
                        nc.tensor.matmul(psc, ct["fbd2c"], hrh,
                                         start=True, stop=False)
                        nc.tensor.matmul(psc, ct["ibd"], ct["g44"],
                                         start=False, stop=True)
                        u = lp.tile([128, 2 * BW], fp16, tag="u")
                        for h in range(2):
                            sl = slice(h * BW, (h + 1) * BW)
                            nc.vector.scalar_tensor_tensor(
                                u[:, sl], psc[:, sl],
                                ct["gI"][:, p2 + h:p2 + h + 1], r1[:, sl],
                                op0=OP.add, op1=OP.mult)
                        usq = lp.tile([128, 2 * BW], fp16, tag="usq")
                        nc.gpsimd.tensor_mul(usq, u, u)
                        nc.tensor.matmul(ps64,
                                         ct["wfinA"][:, 128 * k:128 * k + 128],
                                         usq, start=(k == 0), stop=False)
                        nc.tensor.matmul(ps64,
                                         ct["wfinB"][:, 128 * k:128 * k + 128],
                                         u, start=False, stop=(k == KSUP - 1))
                    else:
                        rin = lp.tile([128, 2 * BW], fp16, tag="rin")
                        for h in range(2):
                            sl = slice(h * BW, (h + 1) * BW)
                            nc.vector.scalar_tensor_tensor(
                                rin[:, sl], ct["f4b"],
                                ct["fIb"][:, p2 + h:p2 + h + 1], r1[:, sl],
                                op0=OP.add, op1=OP.mult)
                        hr = lp.tile([128, 2 * BW], fp16, tag="hr")
                        nc.scalar.activation(hr, rin, AF.Relu,
                                             bias=fb1r, scale=1.0)
                        psc = psC.tile([128, 2 * BW], fp32, tag="psC")
                        nc.tensor.matmul(psc, ct["fbd2c"], hr,
                                         start=True, stop=True)
                        gsc = lp.tile([128, 2 * BW], fp16, tag="gsc")
                        for h in range(2):
                            sl = slice(h * BW, (h + 1) * BW)
                            nc.vector.scalar_tensor_tensor(
                                gsc[:, sl], ct["g4"],
                                ct["gI"][:, p2 + h:p2 + h + 1], r1[:, sl],
                                op0=OP.add, op1=OP.mult)
                        yc = lp.tile([128, 2 * BW], fp16, tag="yc")
                        nc.vector.scalar_tensor_tensor(
                            yc, psc, ycb, gsc, op0=OP.add, op1=OP.add)
                        ysq = lp.tile([128, 2 * BW], fp16, tag="ysq")
                        nc.gpsimd.tensor_mul(ysq, yc, yc)
                        nc.tensor.matmul(ps64,
                                         ct["wfinA"][:, 128 * k:128 * k + 128],
                                         ysq, start=(k == 0), stop=False)
                        nc.tensor.matmul(ps64,
                                         ct["wfinB"][:, 128 * k:128 * k + 128],
                                         yc, start=False, stop=(k == KSUP - 1))

                # super tail: 16 rows of output from packed scalars
                r2p = sup.tile([32, 2 * BW], fp32, tag="r2p")
                act_raw(r2p, ps64[0:32, :], AF.Rsqrt, epsK[0:32, :], 1.0 / D)
                tmp = sup.tile([32, 2 * BW], fp32, tag="tmp")
                nc.vector.scalar_tensor_tensor(
                    tmp, ps64[32:64, :], k2c[0:32, :], r2p,
                    op0=OP.add, op1=OP.mult)
                orow = sup.tile([32, 2 * BW], fp32, tag="orow")
                for h in range(2):
                    sl = slice(h * BW, (h + 1) * BW)
                    nc.vector.scalar_tensor_tensor(
                        orow[:, sl], tmp[:, sl],
                        ct["ddgiP"][0:32, 2 * s + h:2 * s + h + 1],
                        ct["ddgjb"][0:32, :], op0=OP.add, op1=OP.add)

                nk_full = KSUP if s < NSUP - 1 else 6
                for k in range(nk_full):
                    i0 = 16 * s + 2 * k
                    nc.sync.dma_start(
                        out_d[i0:i0 + 2, :].rearrange("h (b c) -> b h c", b=NB),
                        orow[4 * k:4 * k + 4, :].rearrange(
                            "b (h c) -> b h c", h=2))
                if s == NSUP - 1:
                    # row 124 = the half iteration (k=6, h=0)
                    nc.sync.dma_start(
                        out_d[124:125, :].rearrange("o (b c) -> (o b) c", b=NB),
                        orow[24:28, 0:BW])

    nc.compile()
    return nc


def _x4(mat):
    """(1000, 32) -> (128, 250): partition band*32+d, free = token-in-band."""
    return np.ascontiguousarray(
        mat.reshape(NB, BW, D).transpose(0, 2, 1).reshape(128, BW))


def _host_prep(mut1_feat, mut1_ddg, aw1, ab1, aw2, ab2, ag, abt,
               hg, hbt, fw1, fb1, fw2, fb2, fg, fbt, ow, ob):
    f32 = np.float32
    f64 = np.float64
    C = np.eye(D) - np.ones((D, D)) / D
    bd = lambda m: np.kron(np.eye(NB), m).astype(f32)
    tile4 = lambda v: np.tile(np.asarray(v, f64), NB).astype(f32)

    fm = np.asarray(mut1_feat, f64).reshape(M, D)
    aw1_, aw2_, fw1_, fw2_, ow_ = [np.asarray(a, f64) for a in
                                   (aw1, aw2, fw1, fw2, ow)]
    ag_, abt_, hg_, hbt_, fg_, fbt_, ab1_, ab2_, fb1_, fb2_, ob_ = [
        np.asarray(a, f64) for a in
        (ag, abt, hg, hbt, fg, fbt, ab1, ab2, fb1, fb2, ob)]

    # adapter FFN (exact, host fp64), then center so pair sums are centered
    h1 = np.maximum(fm @ aw1_ + ab1_, 0.0)
    y = fm + h1 @ aw2_ + ab2_
    mu = y.mean(-1, keepdims=True)
    v = ((y - mu) ** 2).mean(-1, keepdims=True)
    f = (y - mu) / np.sqrt(v + EPS) * ag_ + abt_
    fc = f - f.mean(-1, keepdims=True)                  # (1000, 32)

    FB = fc @ (hg_[:, None] * fw1_)                     # W1 path, pre-r1
    gz = fc * hg_
    GB = gz - gz.mean(-1, keepdims=True)                # residual path, pre-r1

    cvec = (fg_ * ow_[:, 0])                            # folded LN2 + linear
    fb1r_v = fb1_ + hbt_ @ fw1_
    fast = bool(np.all(fb1r_v == 0.0))
    ycb_v = (fb2_ + hbt_) @ C
    K1 = float(ycb_v @ ycb_v)
    K2 = float(cvec @ ycb_v)
    wfinA = np.zeros((128, 128 * KSUP), f32)
    wfinB = np.zeros((128, 128 * KSUP), f32)
    for kk in range(KSUP):
        for b in range(NB):
            wfinA[32 * b:32 * b + 32, 128 * kk + 4 * kk + b] = 1.0
            wfinB[32 * b:32 * b + 32, 128 * kk + 32 + 4 * kk + b] = cvec
            if fast:
                # cross term 2*ycb.u of sum(yc^2), yc = u + ycb
                wfinB[32 * b:32 * b + 32, 128 * kk + 4 * kk + b] = 2.0 * ycb_v
    obp = float(ob_[0] + fbt_ @ ow_[:, 0])

    ddg = np.asarray(mut1_ddg, f64).reshape(M)
    ddgjb = np.zeros((128, BW), f32)
    for kk in range(KSUP):
        for b in range(NB):
            ddgjb[4 * kk + b, :] = (ddg[BW * b:BW * (b + 1)] + obp)

    f16 = np.float16
    g4x = _x4(GB)
    hbase = {
        "onesbd": bd(np.ones((D, D))).astype(f16),
        "fbd2c": bd(fw2_ @ C).astype(f16),
        "wfinA": wfinA.astype(f16),
        "wfinB": wfinB.astype(f16),
        "x4": _x4(fc).astype(f16),
        "f4b": _x4(FB).astype(f16),
        "g4": g4x.astype(f16),
        "g44": np.concatenate([g4x, g4x], axis=1).astype(f16),
        "ibd": np.eye(128).astype(f16),
    }
    hpack = np.zeros((128, HPW), f16)
    for kname, (o, w) in HOFF.items():
        hpack[:, o:o + w] = hbase[kname]
    base = {
        "ddgjb": ddgjb,
        "vecs": np.stack([
            tile4(fb1r_v),
            tile4(ycb_v),
            np.full(128, EPS),
            np.full(128, EPS + (K1 / D if fast else 0.0)),
            np.full(128, K2 if fast else 0.0),
        ], axis=1).astype(f32),
    }

    in_maps = []
    for c in range(NCORES):
        r0 = c * RPC
        ridx = np.minimum(np.arange(RPAD) + r0, r0 + RPC - 1)
        ddgiP = np.zeros((128, 2 * NSUP), f32)
        for s in range(NSUP):
            for hh in range(2):
                for kk in range(KSUP):
                    for b in range(NB):
                        ddgiP[4 * kk + b, 2 * s + hh] = ddg[
                            ridx[16 * s + 2 * kk + hh]]
        wpack = np.zeros((128, WPW), f32)
        for name, mat in (("fcI", fc), ("fIb", FB), ("gI", GB)):
            o, w = WOFF[name]
            col = np.tile(mat[ridx].T, (NB, 1)).astype(f32)   # (128, 128)
            wpack[:, o:o + w] = col
        for kname in ("ddgjb", "vecs"):
            o, w = WOFF[kname]
            wpack[:, o:o + w] = base[kname]
        o, w = WOFF["ddgiP"]
        wpack[:, o:o + w] = ddgiP
        in_maps.append({"wpack": wpack, "hpack": hpack})
    return (in_maps, fast)


def _sl(name):
    o, w = WOFF[name]
    return o, o + w


def _run(prep, **kw):
    from concourse.bass_utils import run_bass_kernel_spmd
    in_maps, fast = prep
    key = "nc_fast" if fast else "nc_slow"
    if key not in _CACHE:
        _CACHE[key] = _build_program(fast=fast)
    return run_bass_kernel_spmd(_CACHE[key], in_maps,
                                core_ids=list(range(NCORES)), **kw)


def kernel(**inputs):
    res = _run(_host_prep(**inputs))
    rows = np.concatenate([res.results[c]["out"] for c in range(NCORES)], axis=0)
    return rows.reshape(B, L, A, L, A).astype(np.float32)


# revision 24
# speedup vs baseline: 1.3379x; 1.0163x over previous
"""Trainium2 Bass kernel for nn_EpistaticMultiDecoder.

Computes: adapter FFN on (1000,32) features, then for all 1e6 pairs (i,j):
head(LN -> FFN -> Linear(32,1)) of f[i]+f[j], plus ddg[i]+ddg[j].

Strategy: shard rows i across 8 cores (125 rows each, padded to 128).
On-chip layout is "x4": SBUF tile (128, N) where partition = band*32 + d
(4 bands of 32 dims), band b holds j-tokens [250b, 250b+250).

The adapter FFN and every affine per-token transform is host-precomputed in
fp64: fc (centered adapter output), FB = fc @ (hg*fw1), GB = centered(fc*hg).
Since LN1's scale r1 is constant across the 32 dims of a band, the W1 and
residual matmuls commute with the r1 multiply, so per-pair work needs only
4 matmuls: band-variance (ones), W2, and two 4-row finals (sum(yc^2), c.yc).
The finals accumulate 8 iterations into disjoint 4-partition slices of one
(64,500) PSUM tile, so the tail (rsqrt, scale, +ddg, DMA) runs once per 8
iterations on packed (32,500) tiles. All LN sqrt/div pairs are single Rsqrt
activations; Rsqrt/Relu/Square/Identity live in one activation table.
"""

import sys

sys.path.insert(0, "/opt/trn_rl_repo")

import numpy as np

B, L, A, D = 1, 50, 20, 32
M = L * A            # 1000 mutation tokens
NB = 4               # partition bands
BW = M // NB         # 250 tokens per band
NCORES = 8
RPC = M // NCORES    # 125 rows per core
RPAD = 128           # padded row count (rows 125..127 duplicate row 124)
NIT = RPAD // 2      # 64 iterations, 2 rows each
KSUP = 8             # iterations packed per "super" (16 rows)
NSUP = NIT // KSUP   # 8 supers
EPS = 1e-5

_CACHE = {}

# packed constant layouts: fp32 wpack + fp16 hpack
_worder = [("fcI", RPAD), ("fIb", RPAD), ("gI", RPAD),
           ("ddgjb", BW), ("vecs", 5), ("ddgiP", 2 * NSUP)]
WOFF = {}
_o = 0
for _n, _w in _worder:
    WOFF[_n] = (_o, _w)
    _o += _w
WPW = _o
_horder = [("x4", BW), ("f4b", BW), ("g4", BW), ("onesbd", 128), ("fbd2c", 128),
           ("wfinA", 128 * KSUP), ("wfinB", 128 * KSUP), ("g44", 2 * BW),
           ("ibd", 128)]
HOFF = {}
_o = 0
for _n, _w in _horder:
    HOFF[_n] = (_o, _w)
    _o += _w
HPW = _o


def _build_program(fast=True):
    from concourse import bacc, mybir
    from concourse.tile import TileContext

    fp32 = mybir.dt.float32
    fp32r = mybir.dt.float32r
    fp16 = mybir.dt.float16
    AF = mybir.ActivationFunctionType
    OP = mybir.AluOpType
    r = lambda ap: ap.bitcast(fp32r)

    nc = bacc.Bacc()

    def act_raw(out, in_, func, bias, scale):
        # nc.scalar.activation body minus the Rsqrt accuracy guard
        eng = nc.scalar
        ins = [eng.lower_ap(in_), eng.lower_ap(bias),
               mybir.ImmediateValue(dtype=fp32, value=scale),
               mybir.ImmediateValue(dtype=fp32, value=0.0)]
        return eng.add_instruction(mybir.InstActivation(
            name=nc.get_next_instruction_name(), func=func,
            ins=ins, outs=[eng.lower_ap(out)]))

    wp = nc.dram_tensor("wpack", [128, WPW], fp32, kind="ExternalInput")
    hp = nc.dram_tensor("hpack", [128, HPW], fp16, kind="ExternalInput")
    out_d = nc.dram_tensor("out", [RPC, M], fp32, kind="ExternalOutput")

    with TileContext(nc) as tc:
        with (
            tc.tile_pool(name="consts", bufs=1) as consts,
            tc.tile_pool(name="lp", bufs=4) as lp,
            tc.tile_pool(name="sup", bufs=3) as sup,
            tc.tile_pool(name="psA", bufs=3, space="PSUM") as psA,
            tc.tile_pool(name="psC", bufs=3, space="PSUM") as psC,
            tc.tile_pool(name="ps64", bufs=2, space="PSUM") as ps64p,
        ):
            wpack = consts.tile_from(wp[:, :], name="wpack")
            hpack = consts.tile_from(hp[:, :], name="hpack")
            ct = {k: wpack[:, o:o + w] for k, (o, w) in WOFF.items()}
            ct.update({k: hpack[:, o:o + w] for k, (o, w) in HOFF.items()})
            x4 = ct["x4"]
            vecs = ct["vecs"]
            fb1r = vecs[:, 0:1]    # tile4(fb1 + hbt @ fw1)
            ycb = vecs[:, 1:2]     # tile4((fb2 + hbt) @ C32)
            epsr = vecs[:, 2:3]    # EPS
            epsK = vecs[:, 3:4]    # EPS + sum(ycb^2)/D
            k2c = vecs[:, 4:5]     # sum(cvec * ycb)

            for s in range(NSUP):
                ps64 = ps64p.tile([128, 2 * BW], fp32, tag="ps64")
                for k in range(KSUP):
                    p2 = 2 * (s * KSUP + k)

                    # LN1 variance: sum over band of (fc_i + fc_j)^2
                    pairsq = lp.tile([128, 2 * BW], fp16, tag="pairsq")
                    for h in range(2):
                        nc.scalar.activation(
                            pairsq[:, h * BW:(h + 1) * BW], x4, AF.Square,
                            bias=ct["fcI"][:, p2 + h:p2 + h + 1], scale=1.0)
                    psa = psA.tile([128, 2 * BW], fp32, tag="psA")
                    nc.tensor.matmul(psa, ct["onesbd"], pairsq,
                                     start=True, stop=True)
                    r1 = lp.tile([128, 2 * BW], fp16, tag="r1")
                    act_raw(r1, psa, AF.Rsqrt, epsr, 1.0 / D)

                    if fast:
                        # fb1' == 0: relu commutes with the r1 scale and the
                        # block-diag W2 matmul. hrhat = relu(FB_i + FB_j),
                        # psc = W2 @ hrhat + (GB_i replicated + GB_j), and
                        # u = (psc + GI_col) * r1; true yc = u + ycb with ycb
                        # folded into the final contraction weights.
                        hrh = lp.tile([128, 2 * BW], fp16, tag="hrh")
                        for h in range(2):
                            nc.vector.tensor_scalar(
                                hrh[:, h * BW:(h + 1) * BW], ct["f4b"],
                                ct["fIb"][:, p2 + h:p2 + h + 1], 0.0,
                                op0=OP.add, op1=OP.max)
                        psc = psC.tile([128, 2 * BW], fp32, tag="psC")
                        nc.tensor.matmul(psc, ct["fbd2c"], hrh,
                                         start=True, stop=False)
                        nc.tensor.matmul(psc, ct["ibd"], ct["g44"],
                                         start=False, stop=True)
                        u = lp.tile([128, 2 * BW], fp16, tag="u")
                        for h in range(2):
                            sl = slice(h * BW, (h + 1) * BW)
                            nc.vector.scalar_tensor_tensor(
                                u[:, sl], psc[:, sl],
                                ct["gI"][:, p2 + h:p2 + h + 1], r1[:, sl],
                                op0=OP.add, op1=OP.mult)
                        usq = lp.tile([128, 2 * BW], fp16, tag="usq")
                        nc.gpsimd.tensor_mul(usq, u, u)
                        nc.tensor.matmul(ps64,
                                         ct["wfinA"][:, 128 * k:128 * k + 128],
                                         usq, start=(k == 0), stop=False)
                        nc.tensor.matmul(ps64,
                                         ct["wfinB"][:, 128 * k:128 * k + 128],
                                         u, start=False, stop=(k == KSUP - 1))
                    else:
                        rin = lp.tile([128, 2 * BW], fp16, tag="rin")
                        for h in range(2):
                            sl = slice(h * BW, (h + 1) * BW)
                            nc.vector.scalar_tensor_tensor(
                                rin[:, sl], ct["f4b"],
                                ct["fIb"][:, p2 + h:p2 + h + 1], r1[:, sl],
                                op0=OP.add, op1=OP.mult)
                        hr = lp.tile([128, 2 * BW], fp16, tag="hr")
                        nc.scalar.activation(hr, rin, AF.Relu,
                                             bias=fb1r, scale=1.0)
                        psc = psC.tile([128, 2 * BW], fp32, tag="psC")
                        nc.tensor.matmul(psc, ct["fbd2c"], hr,
                                         start=True, stop=True)
                        gsc = lp.tile([128, 2 * BW], fp16, tag="gsc")
                        for h in range(2):
                            sl = slice(h * BW, (h + 1) * BW)
                            nc.vector.scalar_tensor_tensor(
                                gsc[:, sl], ct["g4"],
                                ct["gI"][:, p2 + h:p2 + h + 1], r1[:, sl],
                                op0=OP.add, op1=OP.mult)
                        yc = lp.tile([128, 2 * BW], fp16, tag="yc")
                        nc.vector.scalar_tensor_tensor(
                            yc, psc, ycb, gsc, op0=OP.add, op1=OP.add)
                        ysq = lp.tile([128, 2 * BW], fp16, tag="ysq")
                        nc.gpsimd.tensor_mul(ysq, yc, yc)
                        nc.tensor.matmul(ps64,
                                         ct["wfinA"][:, 128 * k:128 * k + 128],
                                         ysq, start=(k == 0), stop=False)
                        nc.tensor.matmul(ps64,
                                         ct["wfinB"][:, 128 * k:128 * k + 128],
                                         yc, start=False, stop=(k == KSUP - 1))

                # super tail: 16 rows of output from packed scalars
                r2p = sup.tile([32, 2 * BW], fp32, tag="r2p")
                act_raw(r2p, ps64[0:32, :], AF.Rsqrt, epsK[0:32, :], 1.0 / D)
                tmp = sup.tile([32, 2 * BW], fp32, tag="tmp")
                nc.vector.scalar_tensor_tensor(
                    tmp, ps64[32:64, :], k2c[0:32, :], r2p,
                    op0=OP.add, op1=OP.mult)
                orow = sup.tile([32, 2 * BW], fp32, tag="orow")
                for h in range(2):
                    sl = slice(h * BW, (h + 1) * BW)
                    nc.vector.scalar_tensor_tensor(
                        orow[:, sl], tmp[:, sl],
                        ct["ddgiP"][0:32, 2 * s + h:2 * s + h + 1],
                        ct["ddgjb"][0:32, :], op0=OP.add, op1=OP.add)

                nk_full = KSUP if s < NSUP - 1 else 6
                for k in range(nk_full):
                    i0 = 16 * s + 2 * k
                    nc.sync.dma_start(
                        out_d[i0:i0 + 2, :].rearrange("h (b c) -> b h c", b=NB),
                        orow[4 * k:4 * k + 4, :].rearrange(
                            "b (h c) -> b h c", h=2))
                if s == NSUP - 1:
                    # row 124 = the half iteration (k=6, h=0)
                    nc.sync.dma_start(
                        out_d[124:125, :].rearrange("o (b c) -> (o b) c", b=NB),
                        orow[24:28, 0:BW])

    nc.compile()
    return nc


def _x4(mat):
    """(1000, 32) -> (128, 250): partition band*32+d, free = token-in-band."""
    return np.ascontiguousarray(
        mat.reshape(NB, BW, D).transpose(0, 2, 1).reshape(128, BW))


def _host_prep(mut1_feat, mut1_ddg, aw1, ab1, aw2, ab2, ag, abt,
               hg, hbt, fw1, fb1, fw2, fb2, fg, fbt, ow, ob):
    f32 = np.float32
    f64 = np.float64
    C = np.eye(D) - np.ones((D, D)) / D
    bd = lambda m: np.kron(np.eye(NB), m).astype(f32)
    tile4 = lambda v: np.tile(np.asarray(v, f64), NB).astype(f32)

    fm = np.asarray(mut1_feat, f64).reshape(M, D)
    aw1_, aw2_, fw1_, fw2_, ow_ = [np.asarray(a, f64) for a in
                                   (aw1, aw2, fw1, fw2, ow)]
    ag_, abt_, hg_, hbt_, fg_, fbt_, ab1_, ab2_, fb1_, fb2_, ob_ = [
        np.asarray(a, f64) for a in
        (ag, abt, hg, hbt, fg, fbt, ab1, ab2, fb1, fb2, ob)]

    # adapter FFN (exact, host fp64), then center so pair sums are centered
    h1 = np.maximum(fm @ aw1_ + ab1_, 0.0)
    y = fm + h1 @ aw2_ + ab2_
    mu = y.mean(-1, keepdims=True)
    v = ((y - mu) ** 2).mean(-1, keepdims=True)
    f = (y - mu) / np.sqrt(v + EPS) * ag_ + abt_
    fc = f - f.mean(-1, keepdims=True)                  # (1000, 32)

    FB = fc @ (hg_[:, None] * fw1_)                     # W1 path, pre-r1
    gz = fc * hg_
    GB = gz - gz.mean(-1, keepdims=True)                # residual path, pre-r1

    cvec = (fg_ * ow_[:, 0])                            # folded LN2 + linear
    fb1r_v = fb1_ + hbt_ @ fw1_
    fast = bool(np.all(fb1r_v == 0.0))
    ycb_v = (fb2_ + hbt_) @ C
    K1 = float(ycb_v @ ycb_v)
    K2 = float(cvec @ ycb_v)
    wfinA = np.zeros((128, 128 * KSUP), f32)
    wfinB = np.zeros((128, 128 * KSUP), f32)
    for kk in range(KSUP):
        for b in range(NB):
            wfinA[32 * b:32 * b + 32, 128 * kk + 4 * kk + b] = 1.0
            wfinB[32 * b:32 * b + 32, 128 * kk + 32 + 4 * kk + b] = cvec
            if fast:
                # cross term 2*ycb.u of sum(yc^2), yc = u + ycb
                wfinB[32 * b:32 * b + 32, 128 * kk + 4 * kk + b] = 2.0 * ycb_v
    obp = float(ob_[0] + fbt_ @ ow_[:, 0])

    ddg = np.asarray(mut1_ddg, f64).reshape(M)
    ddgjb = np.zeros((128, BW), f32)
    for kk in range(KSUP):
        for b in range(NB):
            ddgjb[4 * kk + b, :] = (ddg[BW * b:BW * (b + 1)] + obp)

    f16 = np.float16
    g4x = _x4(GB)
    hbase = {
        "onesbd": bd(np.ones((D, D))).astype(f16),
        "fbd2c": bd(fw2_ @ C).astype(f16),
        "wfinA": wfinA.astype(f16),
        "wfinB": wfinB.astype(f16),
        "x4": _x4(fc).astype(f16),
        "f4b": _x4(FB).astype(f16),
        "g4": g4x.astype(f16),
        "g44": np.concatenate([g4x, g4x], axis=1).astype(f16),
        "ibd": np.eye(128).astype(f16),
    }
    hpack = np.zeros((128, HPW), f16)
    for kname, (o, w) in HOFF.items():
        hpack[:, o:o + w] = hbase[kname]
    base = {
        "ddgjb": ddgjb,
        "vecs": np.stack([
            tile4(fb1r_v),
            tile4(ycb_v),
            np.full(128, EPS),
            np.full(128, EPS + (K1 / D if fast else 0.0)),
            np.full(128, K2 if fast else 0.0),
        ], axis=1).astype(f32),
    }

    in_maps = []
    for c in range(NCORES):
        r0 = c * RPC
        ridx = np.minimum(np.arange(RPAD) + r0, r0 + RPC - 1)
        ddgiP = np.zeros((128, 2 * NSUP), f32)
        for s in range(NSUP):
            for hh in range(2):
                for kk in range(KSUP):
                    for b in range(NB):
                        ddgiP[4 * kk + b, 2 * s + hh] = ddg[
                            ridx[16 * s + 2 * kk + hh]]
        wpack = np.zeros((128, WPW), f32)
        for name, mat in (("fcI", fc), ("fIb", FB), ("gI", GB)):
            o, w = WOFF[name]
            col = np.tile(mat[ridx].T, (NB, 1)).astype(f32)   # (128, 128)
            wpack[:, o:o + w] = col
        for kname in ("ddgjb", "vecs"):
            o, w = WOFF[kname]
            wpack[:, o:o + w] = base[kname]
        o, w = WOFF["ddgiP"]
        wpack[:, o:o + w] = ddgiP
        in_maps.append({"wpack": wpack, "hpack": hpack})
    return (in_maps, fast)


def _sl(name):
    o, w = WOFF[name]
    return o, o + w


def _run(prep, **kw):
    from concourse.bass_utils import run_bass_kernel_spmd
    in_maps, fast = prep
    key = "nc_fast" if fast else "nc_slow"
    if key not in _CACHE:
        _CACHE[key] = _build_program(fast=fast)
    return run_bass_kernel_spmd(_CACHE[key], in_maps,
                                core_ids=list(range(NCORES)), **kw)


def kernel(**inputs):
    res = _run(_host_prep(**inputs))
    rows = np.concatenate([res.results[c]["out"] for c in range(NCORES)], axis=0)
    return rows.reshape(B, L, A, L, A).astype(np.float32)
